# revision 43
# baseline (speedup 1.0000x reference)
"""AdvancedGraphSAGE (2-layer hetero SAGE + BatchNorm/ReLU) on 8 TRN2 cores.

Strategy (dst-sharded graph parallelism), v4:
  - Nodes sharded 6250/core; each core owns all edges whose dst is local.
    Edge streams sorted by (src-chunk, dst-block, dst), padded per
    (block, chunk) to the cross-core max so all cores run one SPMD program.
  - Src nodes are split into 2 chunks by per-core position (blocks 0-24 /
    25-48). The chunk split keys the layer-0 fp8 x tables, the layer-1 p
    tables, the gather index arrays (shared by both layers), and the layer-1
    halo exchange: TWO AllGathers, so layer-1 chunk-0 gathers/one-hots/
    matmuls overlap the second AllGather.
  - Neighbor mean = one-hot segment matmul: fp8 row gathers (dma_gather,
    int16 idx) x fp16 one-hot (iota == off) * (1/deg) built on DVE; every
    5th layer-1 one-hot is built on gpsimd to relieve the DVE bottleneck.
    One-hots live in two pools: 128 narrow bufs (K=1 tiles, 256B/part) + 16
    wide bufs (block-straddling tiles) — the deep narrow pool lets DVE run
    ~30us ahead of the PE consumers so neither pays per-item cross-engine
    semaphore latency (a shallow shared pool locksteps them).
  - Layer 0 emits type-a means into a persistent tile, then fuses the
    type-s means with the per-block projection so hT completes early.
  - BatchNorm stats: fp16 pairwise tree-folds into a scratch tile on DVE;
    stats exchanged with a tiny AllGather; affine+relu fused on Act.
  - Layer 1: one pass per chunk with contiguous per-block PSUM chains
    (interleaved open accumulation chains in one PSUM bank are illegal:
    only the last-started chain keeps its start contribution). Self term
    rides chunk-0 chains, bias enters via an exact eye-matmul closing each
    final-chunk chain; per-chunk partials summed on DVE at the end.
  - Host-side degree-balanced node relabeling trims slot padding.
All matmul operands fp16/fp8 (PSUM accumulates fp32); everything else fp32.
"""
import sys

if "/opt/trn_rl_repo" not in sys.path:
    sys.path.insert(0, "/opt/trn_rl_repo")

import numpy as np
from contextlib import ExitStack

NCORES = 8
N, E, D, H, C = 50000, 600000, 128, 128, 16
NLOC = N // NCORES
BLK = 128
NB = (NLOC + BLK - 1) // BLK          # 49 blocks; last is 106 wide
NPAD = NB * BLK                        # 6272
HALF = 25000                           # old half (balance perm only)
import os as _os
_CHCFG = _os.environ.get("KCHUNKS", "25,24")
CHB = [int(v) for v in _CHCFG.split(",")]   # blocks per chunk
NCH = len(CHB)
assert sum(CHB) == NB
CB0 = [sum(CHB[:i]) for i in range(NCH)]
CUTS = [CHB[i] * BLK if i < NCH - 1 else NLOC - CB0[i] * BLK
        for i in range(NCH)]
CPOS = [CB0[i] * BLK for i in range(NCH)]
CN = [NCORES * c for c in CUTS]
GB = 64                                # L0 gather batch (128-edge tiles)
GB1 = int(_os.environ.get("KGB1", "64"))   # L1 gather batch
EPS = 1e-5
KMAX = 4                               # max dst blocks one tile may straddle
OFFPAD = 999.0                         # pad off; > 128*KMAX so iota never hits

_CACHE = {}
_DDS = 16384
_NSWQ = 4
_G0BUFS = 6
_G1BUFS = 8
_OHBUFS = int(__import__("os").environ.get("KOHB", "144"))
_OHKBUFS = int(__import__("os").environ.get("KOHKB", "10"))
_POOLOH = int(__import__("os").environ.get("KPOOLOH", "5"))


def _chunk_of(n):
    pos = n % NLOC
    return np.searchsorted(np.asarray(CPOS[1:]), pos, side="right").astype(np.int64)


def _row_of(n, h):
    """Row of node n in its chunk table."""
    core = n // NLOC
    pos = n % NLOC
    cuts = np.asarray(CUTS)[h]
    cpos = np.asarray(CPOS)[h]
    return core * cuts + (pos - cpos)


def _prep_type(src, dst, core):
    """Per-core, per-edge-type segments: dict[(block, chunk)] -> (src, off)."""
    lo = core * NLOC
    m = (dst >= lo) & (dst < lo + NLOC)
    esrc = src[m].astype(np.int64)
    eoff = (dst[m] - lo).astype(np.int64)
    blk = eoff // BLK
    half = _chunk_of(esrc)
    order = np.argsort((half * NB + blk) * (NLOC + 1) + eoff, kind="stable")
    esrc, eoff, blk, half = esrc[order], eoff[order], blk[order], half[order]
    segs = {}
    for h in range(NCH):
        for b in range(NB):
            sel = (blk == b) & (half == h)
            segs[(b, h)] = (esrc[sel], eoff[sel])
    return segs


def _entry_plan(nslots):
    """Shared (cross-core) plan. Streams are chunk-major: for h in (0,1),
    blocks b=0..NB-1 packed back-to-back, each (b,h) segment padded to the
    cross-core max slot count. 128-slot tiles may straddle blocks."""
    Th = {}
    tiles = []
    entries = {b: [] for b in range(NB)}
    for h in range(NCH):
        sblk = []
        for b in range(NB):
            ns = nslots[(b, h)]
            if ns:
                sblk.append(np.full(ns, b, np.int64))
        sblk = np.concatenate(sblk) if sblk else np.zeros(1, np.int64)
        T = (len(sblk) + 127) // 128
        pad = T * 128 - len(sblk)
        if pad:
            sblk = np.concatenate([sblk, np.full(pad, sblk[-1], np.int64)])
        Th[h] = T
        for j in range(T):
            bs = np.unique(sblk[j * 128:(j + 1) * 128])
            b0, K = int(bs[0]), int(bs[-1] - bs[0]) + 1
            assert K <= KMAX, f"tile straddles {K} blocks"
            tiles.append((h, j, b0, K))
    for col, (h, j, b0, K) in enumerate(tiles):
        for b in range(b0, b0 + K):
            entries[b].append((col, h, j, b - b0))
    return Th, tiles, entries


def _build_core_arrays(segs, nslots, tiles, wglob, lo):
    """Per-core slot-stream arrays. idx16 per chunk; off/w columns per TILE
    (off relative to the tile's first covered block b0; pads OFFPAD/0)."""
    idx16 = {}
    soff_h, sw_h = {}, {}
    for h in range(NCH):
        sidx, soff, sw = [], [], []
        for b in range(NB):
            ns = nslots[(b, h)]
            if ns == 0:
                continue
            esrc, eoff = segs[(b, h)]
            ne = len(esrc)
            bi = np.zeros(ns, np.int16)
            bo = np.full(ns, -1.0, np.float32)      # -1 marks a pad slot
            bw = np.zeros(ns, np.float32)
            bi[:ne] = _row_of(esrc, np.full(ne, h)).astype(np.int16)
            bo[:ne] = eoff.astype(np.float32)       # absolute offset in core
            bw[:ne] = wglob[eoff + lo].astype(np.float32)
            sidx.append(bi); soff.append(bo); sw.append(bw)
        if not sidx:
            sidx = [np.zeros(1, np.int16)]
            soff = [np.full(1, -1.0, np.float32)]
            sw = [np.zeros(1, np.float32)]
        sidx = np.concatenate(sidx)
        soff = np.concatenate(soff)
        sw = np.concatenate(sw)
        T = (len(sidx) + 127) // 128
        pad = T * 128 - len(sidx)
        if pad:
            sidx = np.concatenate([sidx, np.zeros(pad, np.int16)])
            soff = np.concatenate([soff, np.full(pad, -1.0, np.float32)])
            sw = np.concatenate([sw, np.zeros(pad, np.float32)])
        a = np.zeros((16, T * 8), np.int16)
        i = np.arange(T * 128)
        a[i % 16, i // 16] = sidx
        idx16[h] = np.tile(a, (8, 1))
        soff_h[h] = soff
        sw_h[h] = sw
    off_cols, w_cols = [], []
    for (h, j, b0, K) in tiles:
        so = soff_h[h][j * 128:(j + 1) * 128]
        swc = sw_h[h][j * 128:(j + 1) * 128]
        oc = np.full(128, OFFPAD, np.float32)
        wc = np.zeros(128, np.float32)
        real = so >= 0
        oc[real] = so[real] - b0 * BLK
        wc[real] = swc[real]
        if real.any():
            assert oc[real].min() >= 0 and oc[real].max() < K * BLK
        off_cols.append(oc)
        w_cols.append(wc)
    out = {f"idx{h}": idx16[h] for h in range(NCH)}
    out["off"] = np.stack(off_cols, axis=1)
    out["w"] = np.stack(w_cols, axis=1)
    return out


def _prepare(inputs):
    """Host preprocessing: sharding, sorting, padding, weight combining."""
    import ml_dtypes

    x = np.asarray(inputs["x"], np.float32)
    edges = {}
    for t, (ks, kd) in (("s", ("sim_src", "sim_dst")),
                        ("a", ("anc_src", "anc_dst"))):
        edges[t] = (np.asarray(inputs[ks]).astype(np.int64),
                    np.asarray(inputs[kd]).astype(np.int64))

    wglob = {}
    for t in ("s", "a"):
        deg = np.bincount(edges[t][1], minlength=N).astype(np.float32)
        wglob[t] = 1.0 / np.maximum(deg, 1.0)

    per_core_segs = {t: [_prep_type(*edges[t], c) for c in range(NCORES)]
                     for t in ("s", "a")}
    nslots = {}
    for t in ("s", "a"):
        ns = {}
        for b in range(NB):
            for h in range(NCH):
                ns[(b, h)] = max(len(per_core_segs[t][c][(b, h)][0])
                                 for c in range(NCORES))
            if sum(ns[(b, h)] for h in range(NCH)) == 0:
                ns[(b, 0)] = 1
        nslots[t] = ns

    struct = {}
    core_arrays = {t: [] for t in ("s", "a")}
    for t in ("s", "a"):
        Th, tiles, entries = _entry_plan(nslots[t])
        struct[t] = {"nslots": nslots[t],
                     "Th": {h: Th[h] for h in range(NCH)},
                     "tiles": tiles, "entries": entries, "T": len(tiles),
                     "K": {(h, j): K for (h, j, b0, K) in tiles}}
        for c in range(NCORES):
            arr = _build_core_arrays(per_core_segs[t][c], nslots[t], tiles,
                                     wglob[t], c * NLOC)
            core_arrays[t].append(arr)

    f16 = np.float16
    wself0 = (0.5 * (np.asarray(inputs["W_self_sim_0"], np.float32)
                     + np.asarray(inputs["W_self_anc_0"], np.float32))).astype(f16)
    wn0s = (0.5 * np.asarray(inputs["W_neigh_sim_0"], np.float32)).astype(f16)
    wn0a = (0.5 * np.asarray(inputs["W_neigh_anc_0"], np.float32)).astype(f16)
    wself1 = (0.5 * (np.asarray(inputs["W_self_sim_1"], np.float32)
                     + np.asarray(inputs["W_self_anc_1"], np.float32))).astype(f16)
    wn1cat = np.concatenate(
        [0.5 * np.asarray(inputs["W_neigh_sim_1"], np.float32),
         0.5 * np.asarray(inputs["W_neigh_anc_1"], np.float32)], axis=1
    ).astype(f16)                                   # [128, 32]
    bias1 = np.broadcast_to(
        0.5 * (np.asarray(inputs["b_sim_1"], np.float32)
               + np.asarray(inputs["b_anc_1"], np.float32)), (128, C)
    ).astype(f16).copy()
    gamma = np.asarray(inputs["bn_gamma_0"], np.float32).reshape(128, 1).copy()
    beta = np.asarray(inputs["bn_beta_0"], np.float32).reshape(128, 1).copy()

    # fp8 x tables at 256B stride (payload = first 128 cols), chunk split
    x8 = np.zeros((N, 256), ml_dtypes.float8_e4m3fn)
    x8[:, :D] = x.astype(ml_dtypes.float8_e4m3fn)
    ids = np.arange(N).reshape(NCORES, NLOC)
    xch = [x8[ids[:, CPOS[h]:CPOS[h] + CUTS[h]].ravel()].view(np.uint8).copy()
           for h in range(NCH)]

    in_maps = []
    for c in range(NCORES):
        xlT = np.zeros((128, NPAD), f16)
        xlT[:, :NLOC] = x[c * NLOC:(c + 1) * NLOC].T.astype(f16)
        im = {
            **{f"x8_{h}": xch[h] for h in range(NCH)}, "xlT": xlT,
            "wself0": wself0, "wn0s": wn0s, "wn0a": wn0a,
            "wself1": wself1, "wn1cat": wn1cat,
            "bias1": bias1, "eye": np.eye(128, dtype=f16),
            "zero16": np.zeros((128, C), f16),
            "gamma": gamma, "beta": beta,
        }
        for t in ("s", "a"):
            arr = core_arrays[t][c]
            for h in range(NCH):
                im[f"idx_{t}_{h}"] = arr[f"idx{h}"]
            im[f"off_{t}"] = arr["off"]
            im[f"w_{t}"] = arr["w"]
        in_maps.append(im)
    return struct, in_maps


def _raw_gather(nc, out_ap, in_ap, idxs_ap, num_idxs, num_idxs_reg, elem_size,
                queue_num=0):
    """dma_gather without the elem_size%256 wrapper assert (table row
    stride must still be a multiple of 256B)."""
    import concourse.mybir as mybir
    from concourse import ap_utils
    from concourse.bass import round_up_to_multiple, exact_div

    eng = nc.gpsimd
    elem_step = in_ap.ap[0][0]
    stride_bytes = elem_step * mybir.dt.size(in_ap.dtype)
    stride_bytes_256 = exact_div(stride_bytes, 256)
    assert stride_bytes_256 < 256
    assert ap_utils.ap_is_contiguous(in_ap.ap[1:])
    assert ap_utils.ap_is_contiguous(out_ap.ap[1:])
    assert ap_utils.ap_is_contiguous(idxs_ap.ap[1:])
    assert in_ap.ap[-1][1] == out_ap.ap[-1][1] == elem_size
    assert out_ap.ap[0][1] * out_ap.ap[1][1] == round_up_to_multiple(num_idxs, 128)
    _in_ap = eng.lower_ap_dma(in_ap, for_custom_bir_dma=True)
    _idxs_ap = eng.lower_ap(idxs_ap)
    _out_ap = eng.lower_ap(out_ap)
    return eng.add_instruction(
        mybir.InstDMAGatherAnt(
            name=eng.bass.get_next_instruction_name(),
            ins=[*_in_ap, _idxs_ap, eng.lower_val_access(eng.to_reg(num_idxs_reg))],
            outs=[_out_ap],
            transpose=False,
            num_idxs=num_idxs,
            elem_size=elem_size,
            stride_bytes_256=stride_bytes_256,
            gen_mode=0,
            single_packet=False,
            queue_num=queue_num,
            sbuf_tokens_per_rank=0,
            sbuf_free_dim_per_rank=0,
            sbuf_free_dim_pad_per_rank=0,
            sbuf_byte_offset=0,
        )
    )


def _build(struct):
    import concourse.bacc as bacc
    import concourse.mybir as mybir
    import concourse.tile as tile

    f16, f32 = mybir.dt.float16, mybir.dt.float32
    f8 = mybir.dt.float8e4
    nc = bacc.Bacc(None, num_devices=NCORES, dynamic_dma_scratch_size=_DDS,
                   num_swdge_queues=_NSWQ)

    din = {}
    def inp(name, shape, dtype):
        din[name] = nc.dram_tensor(name, shape, dtype, kind="ExternalInput")
        return din[name]

    for h in range(NCH):
        inp(f"x8_{h}", [CN[h], 256], f8)
    inp("xlT", [128, NPAD], f16)
    inp("wself0", [128, 128], f16)
    inp("wn0s", [128, 128], f16)
    inp("wn0a", [128, 128], f16)
    inp("wself1", [128, C], f16)
    inp("wn1cat", [128, 2 * C], f16)
    inp("bias1", [128, C], f16)
    inp("eye", [128, 128], f16)
    inp("zero16", [128, C], f16)
    inp("gamma", [128, 1], f32)
    inp("beta", [128, 1], f32)
    for t in ("s", "a"):
        st = struct[t]
        for h in range(NCH):
            inp(f"idx_{t}_{h}", [128, max(st["Th"][h], 1) * 8], mybir.dt.int16)
        inp(f"off_{t}", [128, st["T"]], f32)
        inp(f"w_{t}", [128, st["T"]], f32)
    out_d = nc.dram_tensor("out", [128, NB * C], f32, kind="ExternalOutput")
    import os
    _dbg = os.environ.get("KDBG") == "1"
    if _dbg:
        dbg_h = nc.dram_tensor("dbg_h", [128, NPAD], f16, kind="ExternalOutput")
        pass

    C2 = 2 * C
    BWL = NLOC - (NB - 1) * BLK            # 106: width of last block

    with tile.TileContext(nc) as tc, ExitStack() as ctx:
        per = ctx.enter_context(tc.tile_pool(name="per", bufs=1))
        gp = ctx.enter_context(tc.tile_pool(name="gp", bufs=_G0BUFS))
        ohp = ctx.enter_context(tc.tile_pool(name="ohp", bufs=_OHBUFS))
        ohpK = ctx.enter_context(tc.tile_pool(name="ohpK", bufs=_OHKBUFS))
        sm = ctx.enter_context(tc.tile_pool(name="sm", bufs=2))
        ps = ctx.enter_context(tc.tile_pool(name="ps", bufs=2, space="PSUM"))
        dr = ctx.enter_context(tc.tile_pool(name="dr", bufs=1, space="DRAM"))

        load_engs = [nc.sync, nc.scalar]
        _lrot = [0]
        def load(name):
            d = din[name]
            t = per.tile(list(d.shape), d.dtype, tag=name)
            load_engs[_lrot[0] % 2].dma_start(out=t[:], in_=d[:, :])
            _lrot[0] += 1
            return t

        # idx/off/w for type "a" first (feeds the first gathers), then rest
        sb = {k: load(k) for k in
              (["idx_a_0", "off_a", "w_a"]
               + [f"idx_a_{h}" for h in range(1, NCH)]
               + ["idx_s_0", "off_s", "w_s"]
               + [f"idx_s_{h}" for h in range(1, NCH)]
               + ["xlT", "wself0", "wn0s", "wn0a", "wself1", "wn1cat",
                  "bias1", "eye", "zero16", "gamma", "beta"])}
        iota = per.tile([128, KMAX * BLK], f16, tag="iota")
        ioti = per.tile([128, KMAX * BLK], mybir.dt.int16, tag="ioti")
        nc.gpsimd.iota(ioti[:], pattern=[[1, KMAX * BLK]], base=0,
                       channel_multiplier=0)
        nc.vector.tensor_copy(out=iota[:], in_=ioti[:])

        nireg_cache = {}
        def nireg(v):
            if v not in nireg_cache:
                nireg_cache[v] = nc.gpsimd.to_reg(v)
            return nireg_cache[v]

        hT = per.tile([128, NPAD], f16, tag="hT")

        ploc = [dr.tile([128, CHB[h] * C2], f16, tag=f"ploc{h}", name=f"ploc{h}")
                for h in range(NCH)]
        pfull = [dr.tile([NCORES * 128, CHB[h], C2], f16, tag=f"pfull{h}", name=f"pfull{h}")
                 for h in range(NCH)]
        pcat = [dr.tile([CN[h], 128], f16, tag=f"pcat{h}", name=f"pcat{h}")
                for h in range(NCH)]
        bnin = dr.tile([128, 2], f32)
        bnout = dr.tile([NCORES * 128, 2], f32)

        class Stream:
            """Gather + one-hot machinery for one (layer, type)."""

            def __init__(self, layer, t, gtag, gbufs):
                st = struct[t]
                self.t, self.layer, self.st = t, layer, st
                self.idx = {h: sb[f"idx_{t}_{h}"] for h in range(NCH)}
                self.tot = dict(st["Th"])
                if layer == 0:
                    self.tabs = {h: din[f"x8_{h}"][0:CN[h], 0:128]
                                 for h in range(NCH)}
                    self.esz, self.gdt, self.gw = 128, f8, 128
                else:
                    c0 = 0 if t == "s" else C
                    self.tabs = {h: pcat[h][0:CN[h], c0:c0 + C]
                                 for h in range(NCH)}
                    self.esz, self.gdt, self.gw = C, f16, C
                self.gtag, self.gbufs = gtag, gbufs
                self.gb = GB if layer == 0 else GB1
                self.gbuf = {h: [] for h in range(NCH)}
                self.emitted = {h: 0 for h in range(NCH)}
                self.oh_of = {}

            def ensure(self, h, batch):
                while self.emitted[h] <= batch:
                    k = self.emitted[h]
                    GBs = self.gb
                    nb_t = min(GBs, self.tot[h] - k * GBs)
                    g = gp.tile([128, GBs, self.gw], self.gdt, tag=self.gtag,
                                bufs=self.gbufs)
                    _raw_gather(nc, g[:, :nb_t, :], self.tabs[h],
                                self.idx[h][:, k * GBs * 8:(k * GBs + nb_t) * 8],
                                nb_t * 128, nireg(nb_t * 128), self.esz,
                                queue_num=_qrot[0] % _NSWQ)
                    _qrot[0] += 1
                    self.gbuf[h].append(g)
                    self.emitted[h] += 1

            def get(self, col, h, j):
                """-> (oh_tile, K, g_tile, slot); oh covers K*BLK offsets."""
                K = self.st["K"][(h, j)]
                batch, slot = j // self.gb, j % self.gb
                self.ensure(h, batch)
                g = self.gbuf[h][batch]
                key = (h, j)
                hit = self.oh_of.get(key)
                if hit is not None:
                    oh0, stamp, wide = hit
                    seq, lim = ((_oh_seqK, _OHKBUFS) if wide
                                else (_oh_seq, _OHBUFS))
                    if seq[0] - stamp < lim - 1:
                        return oh0, K, g, slot
                wide = K > 1
                if wide:
                    oh = ohpK.tile([128, KMAX * BLK], f16, tag="ohK")
                    _oh_seqK[0] += 1
                    stamp = _oh_seqK[0]
                else:
                    oh = ohp.tile([128, BLK], f16, tag="oh")
                    _oh_seq[0] += 1
                    stamp = _oh_seq[0]
                eng = (nc.gpsimd if (_POOLOH and self.layer == 1
                                     and _oh_seq[0] % _POOLOH == 0)
                       else nc.vector)
                eng.tensor_scalar(
                    out=oh[:, 0:K * BLK], in0=iota[:, 0:K * BLK],
                    scalar1=sb[f"off_{self.t}"][:, col:col + 1],
                    scalar2=sb[f"w_{self.t}"][:, col:col + 1],
                    op0=mybir.AluOpType.is_equal,
                    op1=mybir.AluOpType.mult)
                self.oh_of[key] = (oh, stamp, wide)
                return oh, K, g, slot

        _oh_seq = [0]
        _oh_seqK = [0]
        _qrot = [0]

        # ---- layer 0: type-a means (full tile), then type-s + proj fused ----
        mean_a = per.tile([128, NPAD], f16, tag="mean_a")
        srm = Stream(0, "a", "g0", _G0BUFS)
        for b in range(NB):
            ents = srm.st["entries"][b]
            cols = slice(b * BLK, (b + 1) * BLK)
            if not ents:
                nc.gpsimd.memset(mean_a[:, cols], 0.0)
                continue
            psum = ps.tile([128, BLK], f32, tag="pb", bufs=6)
            for k, (col, h, j, hk) in enumerate(ents):
                oh, K, g, slot = srm.get(col, h, j)
                nc.tensor.matmul(
                    out=psum[:], lhsT=g[:, slot, :],
                    rhs=oh[:, hk * BLK:(hk + 1) * BLK],
                    start=(k == 0), stop=(k == len(ents) - 1))
            nc.scalar.activation(mean_a[:, cols], psum[:],
                                 mybir.ActivationFunctionType.Copy)
        srm = Stream(0, "s", "g0", _G0BUFS)
        for b in range(NB):
            ents = srm.st["entries"][b]
            cols = slice(b * BLK, (b + 1) * BLK)
            mean = sm.tile([128, BLK], f16, tag="mean_s", bufs=6)
            if not ents:
                nc.gpsimd.memset(mean[:], 0.0)
            else:
                psum = ps.tile([128, BLK], f32, tag="pb", bufs=6)
                for k, (col, h, j, hk) in enumerate(ents):
                    oh, K, g, slot = srm.get(col, h, j)
                    nc.tensor.matmul(
                        out=psum[:], lhsT=g[:, slot, :],
                        rhs=oh[:, hk * BLK:(hk + 1) * BLK],
                        start=(k == 0), stop=(k == len(ents) - 1))
                nc.scalar.activation(mean[:], psum[:],
                                     mybir.ActivationFunctionType.Copy)
            po = ps.tile([128, BLK], f32, tag="pb", bufs=6)
            nc.tensor.matmul(out=po[:], lhsT=sb["wself0"][:],
                             rhs=sb["xlT"][:, cols], start=True, stop=False)
            nc.tensor.matmul(out=po[:], lhsT=sb["wn0s"][:], rhs=mean[:],
                             start=False, stop=False)
            nc.tensor.matmul(out=po[:], lhsT=sb["wn0a"][:],
                             rhs=mean_a[:, cols], start=False, stop=True)
            nc.scalar.activation(hT[:, cols], po[:],
                                 mybir.ActivationFunctionType.Copy)

        # ------- batchnorm: fp16 tree-fold stats on scratch -------
        bnv = per.tile([128, 2], f32, tag="bnv")
        nc.vector.tensor_tensor(out=mean_a[:, 0:NPAD], in0=hT[:, 0:NPAD],
                                in1=hT[:, 0:NPAD], op=mybir.AluOpType.mult)
        for col, first_in in ((1, mean_a), (0, hT)):
            w_ = NPAD // 2
            nc.vector.tensor_tensor(out=mean_a[:, 0:w_], in0=first_in[:, 0:w_],
                                    in1=first_in[:, w_:2 * w_],
                                    op=mybir.AluOpType.add)
            while w_ > 784:
                w_ //= 2
                nc.vector.tensor_tensor(out=mean_a[:, 0:w_], in0=mean_a[:, 0:w_],
                                        in1=mean_a[:, w_:2 * w_],
                                        op=mybir.AluOpType.add)
            nc.vector.tensor_reduce(out=bnv[:, col:col + 1], in_=mean_a[:, 0:w_],
                                    axis=mybir.AxisListType.X,
                                    op=mybir.AluOpType.add)
        nc.sync.dma_start(out=bnin[:], in_=bnv[:])
        nc.gpsimd.collective_compute(
            "AllGather", mybir.AluOpType.bypass,
            replica_groups=[list(range(NCORES))],
            ins=[bnin[:].opt()], outs=[bnout[:, :].opt()])
        bng8 = per.tile([128, NCORES, 2], f32, tag="bng8")
        bnga = bnout[0:NCORES * 128, :].copy()
        bnga.ap = mybir.VecI64Pair([[2, 128], [256, NCORES], [1, 2]])
        nc.sync.dma_start(out=bng8[:, :, :], in_=bnga)
        # post-exchange BN math on DVE: DVE is waiting for matmul consumers
        # at this point anyway (48-buf one-hot runway), so the stall is free
        bng = per.tile([128, 2], f32, tag="bng")
        nc.vector.tensor_reduce(out=bng[:, 0:1], in_=bng8[:, :, 0],
                                axis=mybir.AxisListType.X,
                                op=mybir.AluOpType.add)
        nc.vector.tensor_reduce(out=bng[:, 1:2], in_=bng8[:, :, 1],
                                axis=mybir.AxisListType.X,
                                op=mybir.AluOpType.add)
        mu = per.tile([128, 1], f32, tag="mu")
        ex2 = per.tile([128, 1], f32, tag="ex2")
        var = per.tile([128, 1], f32, tag="var")
        sd = per.tile([128, 1], f32, tag="sd")
        rs = per.tile([128, 1], f32, tag="rs")
        av = per.tile([128, 1], f32, tag="av")
        bv = per.tile([128, 1], f32, tag="bv")
        tmp = per.tile([128, 1], f32, tag="tmp")
        nc.vector.tensor_scalar_mul(mu[:], bng[:, 0:1], 1.0 / N)
        nc.vector.tensor_scalar_mul(ex2[:], bng[:, 1:2], 1.0 / N)
        nc.vector.tensor_tensor(out=tmp[:], in0=mu[:], in1=mu[:],
                                op=mybir.AluOpType.mult)
        nc.vector.tensor_tensor(out=var[:], in0=ex2[:], in1=tmp[:],
                                op=mybir.AluOpType.subtract)
        nc.vector.tensor_scalar_add(var[:], var[:], EPS)
        nc.scalar.activation(sd[:], var[:], mybir.ActivationFunctionType.Sqrt)
        nc.vector.reciprocal(rs[:], sd[:])
        nc.vector.tensor_tensor(out=av[:], in0=sb["gamma"][:], in1=rs[:],
                                op=mybir.AluOpType.mult)
        nc.vector.tensor_tensor(out=tmp[:], in0=av[:], in1=mu[:],
                                op=mybir.AluOpType.mult)
        nc.vector.tensor_tensor(out=bv[:], in0=sb["beta"][:], in1=tmp[:],
                                op=mybir.AluOpType.subtract)
        # ---- affine+relu, project, and halo-exchange per chunk ----
        pn_all = per.tile([128, NB, C2], f16, tag="pn_all")
        for h in range(NCH):
            b0, b1 = CB0[h], CB0[h] + CHB[h]
            nc.scalar.activation(hT[:, b0 * BLK:b1 * BLK],
                                 hT[:, b0 * BLK:b1 * BLK],
                                 mybir.ActivationFunctionType.Relu,
                                 scale=av[:], bias=bv[:])
            done = b0
            while done < b1:
                nb_ = min(16, b1 - done)
                ppt = ps.tile([128, 16, C2], f32, tag="pp", bufs=2)
                for i in range(nb_):
                    b = done + i
                    cols = slice(b * BLK, (b + 1) * BLK)
                    nc.tensor.matmul(out=ppt[:, i, :], lhsT=hT[:, cols],
                                     rhs=sb["wn1cat"][:],
                                     start=True, stop=True)
                nc.scalar.activation(pn_all[:, done:done + nb_, :],
                                     ppt[:, 0:nb_, :],
                                     mybir.ActivationFunctionType.Copy)
                done += nb_
            nc.sync.dma_start(out=ploc[h][:, :], in_=pn_all[:, b0:b1, :])
            nc.gpsimd.collective_compute(
                "AllGather", mybir.AluOpType.bypass,
                replica_groups=[list(range(NCORES))],
                ins=[ploc[h][:].opt()], outs=[pfull[h][:, :, :].opt()])

        # re-stride halo rows into pcat[h] (256B rows, 64B payload)
        engs = [nc.sync, nc.scalar]
        for h in range(NCH):
            NBh = CHB[h]
            RS = NBh * C2
            full = NBh if h < NCH - 1 else NBh - 1
            for c in range(NCORES):
                o = pcat[h][0:CN[h], 0:C2].copy()
                o.offset = c * CUTS[h] * 128
                o.ap = mybir.VecI64Pair([[BLK * 128, full], [128, BLK], [1, C2]])
                i = pfull[h][0:NCORES * 128, 0:NBh, :].copy()
                i.offset = c * 128 * RS
                i.ap = mybir.VecI64Pair([[C2, full], [RS, BLK], [1, C2]])
                engs[c % 2].dma_start(out=o, in_=i)
            if full < NBh:
                for gh in range(2):
                    cg = NCORES // 2
                    o = pcat[h][0:CN[h], 0:C2].copy()
                    o.offset = (gh * cg * CUTS[h] + full * BLK) * 128
                    o.ap = mybir.VecI64Pair([[CUTS[h] * 128, cg], [128, BWL], [1, C2]])
                    i = pfull[h][0:NCORES * 128, 0:NBh, :].copy()
                    i.offset = gh * cg * 128 * RS + full * C2
                    i.ap = mybir.VecI64Pair([[128 * RS, cg], [RS, BWL], [1, C2]])
                    engs[gh].dma_start(out=o, in_=i)

        # ---------- layer 1: one pass per chunk, contiguous PSUM chains ----
        # (interleaved open accumulation chains in one PSUM bank are illegal:
        # only the last-started chain keeps its start contribution)
        srm1 = {t: Stream(1, t, "g1", _G1BUFS) for t in ("s", "a")}
        obs = [per.tile([128, NB, C], f32, tag=f"ob{h}", name=f"ob{h}") for h in range(NCH)]
        ents1 = {b: [(t,) + e for t in ("s", "a")
                     for e in srm1[t].st["entries"][b]] for b in range(NB)}
        for h in range(NCH):
            for b in range(NB):
                eh = [e for e in ents1[b] if e[2] == h]
                if h not in (0, NCH - 1) and not eh:
                    nc.gpsimd.memset(obs[h][:, b, :], 0.0)
                    continue
                cols = slice(b * BLK, (b + 1) * BLK)
                pst = ps.tile([128, BLK], f32, tag="pb", bufs=6)
                psum = pst[:, 0:C]
                started = False
                if h == 0:
                    nc.tensor.matmul(out=psum, lhsT=hT[:, cols],
                                     rhs=sb["wself1"][:],
                                     start=True, stop=False)
                    started = True
                for k, (t, col, hh, j, hk) in enumerate(eh):
                    oh, K, g, slot = srm1[t].get(col, hh, j)
                    nc.tensor.matmul(
                        out=psum, lhsT=oh[:, hk * BLK:(hk + 1) * BLK],
                        rhs=g[:, slot, :], start=not started, stop=False)
                    started = True
                nc.tensor.matmul(out=psum, lhsT=sb["eye"][:],
                                 rhs=sb["bias1"][:] if h == NCH - 1
                                 else sb["zero16"][:],
                                 start=not started, stop=True)
                nc.scalar.activation(obs[h][:, b, :], psum,
                                     mybir.ActivationFunctionType.Copy)
        for lo, hi in ((0, 28), (28, NB)):
            for h in range(1, NCH):
                nc.vector.tensor_tensor(out=obs[0][:, lo:hi, :],
                                        in0=obs[0][:, lo:hi, :],
                                        in1=obs[h][:, lo:hi, :],
                                        op=mybir.AluOpType.add)
            nc.sync.dma_start(out=out_d[:, lo * C:hi * C],
                              in_=obs[0][:, lo:hi, :])
        if _dbg:
            nc.sync.dma_start(out=dbg_h[:, :], in_=hT[:, :])
    nc.compile()
    return nc


def _balance_perm(inputs):
    """Half-preserving node relabeling that balances per-(block, src-half,
    type) in-degree across cores, shrinking the cross-core max slot padding.
    Returns pi (node -> new position) with (pi >= HALF) == (node >= HALF)."""
    deg4 = np.zeros((N, 4), np.int64)
    for ti, (ks, kd) in enumerate((("sim_src", "sim_dst"),
                                   ("anc_src", "anc_dst"))):
        s = np.asarray(inputs[ks]).astype(np.int64)
        d = np.asarray(inputs[kd]).astype(np.int64)
        h = (s >= HALF).astype(np.int64)
        np.add.at(deg4, (d, ti * 2 + h), 1)
    pi = np.empty(N, np.int64)
    ncl = NCORES // 2                    # cores per half-group
    orders, ptr = [], [0, 0]
    for half in range(2):
        ids = np.arange(half * HALF, HALF + half * HALF)
        orders.append(ids[np.argsort(-deg4[ids].sum(1), kind="stable")])
    for b in range(NB):
        wb = min(BLK, NLOC - b * BLK)
        pool = []
        for half in range(2):
            pool.append(orders[half][ptr[half]:ptr[half] + ncl * wb])
            ptr[half] += ncl * wb
        cand = np.concatenate(pool)
        cand = cand[np.argsort(-deg4[cand].sum(1), kind="stable")]
        load = np.zeros((NCORES, 4), np.int64)
        cnt = np.zeros(NCORES, np.int64)
        for n in cand:
            g = int(n >= HALF)
            cs = np.arange(g * ncl, g * ncl + ncl)
            open_c = cs[cnt[cs] < wb]
            phi = ((load[open_c] + deg4[n]) ** 2).sum(1)
            c = int(open_c[int(np.argmin(phi))])
            pi[n] = c * NLOC + b * BLK + cnt[c]
            cnt[c] += 1
            load[c] += deg4[n]
    assert ptr[0] == HALF and ptr[1] == HALF
    return pi


def kernel(**inputs):
    from concourse.bass_utils import run_bass_kernel_spmd

    pi = _balance_perm(inputs)
    inv = np.argsort(pi)
    inputs = dict(inputs)
    inputs["x"] = np.asarray(inputs["x"], np.float32)[inv]
    for k in ("sim_src", "sim_dst", "anc_src", "anc_dst"):
        inputs[k] = pi[np.asarray(inputs[k]).astype(np.int64)]
    struct, in_maps = _prepare(inputs)
    key = (tuple(sorted(struct["s"]["nslots"].items())),
           tuple(sorted(struct["a"]["nslots"].items())))
    if key not in _CACHE:
        _CACHE.clear()
        _CACHE[key] = _build(struct)
    nc = _CACHE[key]
    res = run_bass_kernel_spmd(nc, in_maps, core_ids=list(range(NCORES)))
    outs = []
    for c in range(NCORES):
        o = res.results[c]["out"].reshape(128, NB, C)
        outs.append(o.transpose(1, 0, 2).reshape(NPAD, C)[:NLOC])
    return np.concatenate(outs, axis=0)[pi]
